# revision 12
# baseline (speedup 1.0000x reference)
"""Tensor-parallel multi-head attention for Trainium2 (8 NeuronCores).

Problem: B=2, T=2048, E=1024, H=16 heads of dim 64.
  q/k/v = einsum('hei,bte->hbti'); s = q@k^T/sqrt(T); p = softmax(s)
  att = p@v; out = concat_heads(att) @ Wo^T

Sharding: tensor-parallel over heads — 2 heads per core. Each core computes
its heads' attention plus its slice of the output projection (Wo sharded
along its input axis); partial outputs are summed across cores.

Numerics: attention logits have std ~181 (unscaled randn weights), so softmax
is nearly one-hot and the QK^T path needs ~fp32 precision. We use an exact
hi/lo bf16 split (x = hi + lo, dropping only the lo*lo term) for the Q/K
projections and QK^T: 3 bf16 matmuls instead of 1 fp32 matmul (which costs
4x on the PE). Validated: adds ~3.5e-4 relative error. V path / PV / Wo run
in plain bf16 (~3.4e-3 total relative error).
"""

import sys

sys.path.insert(0, "/opt/trn_rl_repo")

import numpy as np
import ml_dtypes

import concourse.bass as bass
import concourse.mybir as mybir
import concourse.tile as tile
from concourse import bacc

BF16 = ml_dtypes.bfloat16
NF16 = np.float16

B, T, E = 2, 2048, 1024
H, I = 16, 64
NCORES = 8
HPC = H // NCORES            # heads per core = 2
BT = B * T                   # 4096
HI = HPC * I                 # 128 = per-core slice of the h*i axis
EC = E // 128                # 8 e-chunks
SCALE = 1.0 / float(np.sqrt(np.float32(T)))

F32 = mybir.dt.float32
BF = mybir.dt.bfloat16
FP16 = mybir.dt.float16

USE_HILO = True


def build_program(use_hilo: bool = USE_HILO) -> bass.Bass:
    nc = bacc.Bacc("TRN2", target_bir_lowering=False, debug=False,
                   num_devices=NCORES)

    # --- DRAM I/O (per-core contents supplied via in_maps) ---
    if use_hilo:
        xh_d = nc.dram_tensor("xh", [E, BT], FP16, kind="ExternalInput")
        xl_d = nc.dram_tensor("xl", [E, BT], FP16, kind="ExternalInput")
        wqh_d = nc.dram_tensor("wqh", [128, EC, HI], FP16, kind="ExternalInput")
        wql_d = nc.dram_tensor("wql", [128, EC, HI], FP16, kind="ExternalInput")
        wkh_d = nc.dram_tensor("wkh", [128, EC, HI], FP16, kind="ExternalInput")
        wkl_d = nc.dram_tensor("wkl", [128, EC, HI], FP16, kind="ExternalInput")
    else:
        xt_d = nc.dram_tensor("xt", [E, BT], F32, kind="ExternalInput")
        xh_d = nc.dram_tensor("xh", [E, BT], FP16, kind="ExternalInput")
        wq_d = nc.dram_tensor("wq", [128, EC, HI], F32, kind="ExternalInput")
        wk_d = nc.dram_tensor("wk", [128, EC, HI], F32, kind="ExternalInput")
    wv_d = nc.dram_tensor("wv", [128, EC, HI], FP16, kind="ExternalInput")
    wo_d = nc.dram_tensor("wo_t", [HI, E], FP16, kind="ExternalInput")
    out_d = nc.dram_tensor("out", [BT, E], F32, kind="ExternalOutput")

    with tile.TileContext(nc) as tc:
        with (
            tc.tile_pool(name="psum", bufs=8, space="PSUM") as psp,
            tc.tile_pool(name="xstream", bufs=4) as xp,
            tc.tile_pool(name="weights", bufs=1) as wp,
            tc.tile_pool(name="persist", bufs=1) as pk,
            tc.tile_pool(name="big", bufs=1) as bigp,
            tc.tile_pool(name="ptile", bufs=3) as ptp,
            tc.tile_pool(name="stats", bufs=8) as stp,
            tc.tile_pool(name="evac", bufs=3) as evp,
        ):
            # --- load weights into SBUF ---
            if use_hilo:
                wqh = wp.tile([128, EC, HI], FP16, tag="wqh")
                wql = wp.tile([128, EC, HI], FP16, tag="wql")
                wkh = wp.tile([128, EC, HI], FP16, tag="wkh")
                wkl = wp.tile([128, EC, HI], FP16, tag="wkl")
                nc.sync.dma_start(wqh[:], wqh_d[:])
                nc.sync.dma_start(wql[:], wql_d[:])
                nc.sync.dma_start(wkh[:], wkh_d[:])
                nc.sync.dma_start(wkl[:], wkl_d[:])
            else:
                wq = wp.tile([128, EC, HI], F32, tag="wq")
                wk = wp.tile([128, EC, HI], F32, tag="wk")
                nc.sync.dma_start(wq[:], wq_d[:])
                nc.sync.dma_start(wk[:], wk_d[:])
            wv = wp.tile([128, EC, HI], FP16, tag="wv")
            nc.sync.dma_start(wv[:], wv_d[:])
            wo = wp.tile([128, E], FP16, tag="wo")
            nc.sync.dma_start(wo[:], wo_d[:])

            # --- persistent activations ---
            if use_hilo:
                Qh = pk.tile([128, BT], BF, tag="Qh")
                Ql = pk.tile([128, BT], BF, tag="Ql")
                Kh = pk.tile([128, BT], BF, tag="Kh")
                Kl = pk.tile([128, BT], BF, tag="Kl")
            else:
                QT = pk.tile([128, BT], F32, tag="QT")
                KT = pk.tile([128, BT], F32, tag="KT")
            V = pk.tile([128, BT // 128, HI], FP16, tag="V")
            OT = pk.tile([128, BT], FP16, tag="OT")

            # ================= Phase 1: QKV projections =================
            # Q^T[i, t] = sum_e W[e, i] * xT[e, t]; t-banks of 512.
            for tb8 in range(BT // 512):
                ts = slice(tb8 * 512, (tb8 + 1) * 512)
                qt_ps = psp.tile([128, 512], F32, tag="ps")
                kt_ps = psp.tile([128, 512], F32, tag="ps")
                vt_ps = psp.tile([128, 512], F32, tag="ps")
                for ec in range(EC):
                    es = slice(ec * 128, (ec + 1) * 128)
                    if use_hilo:
                        xhs = xp.tile([128, 512], BF, tag="xhs")
                        xls = xp.tile([128, 512], BF, tag="xls")
                        nc.gpsimd.dma_start(xhs[:], xh_d[es, ts])
                        nc.gpsimd.dma_start(xls[:], xl_d[es, ts])
                        # Q^T += Wh'xh + Wh'xl + Wl'xh  (and same for K)
                        nc.tensor.matmul(qt_ps[:], wqh[:, ec, :], xhs[:],
                                         start=(ec == 0), stop=False)
                        nc.tensor.matmul(qt_ps[:], wqh[:, ec, :], xls[:],
                                         start=False, stop=False)
                        nc.tensor.matmul(qt_ps[:], wql[:, ec, :], xhs[:],
                                         start=False, stop=(ec == EC - 1))
                        nc.tensor.matmul(kt_ps[:], wkh[:, ec, :], xhs[:],
                                         start=(ec == 0), stop=False)
                        nc.tensor.matmul(kt_ps[:], wkh[:, ec, :], xls[:],
                                         start=False, stop=False)
                        nc.tensor.matmul(kt_ps[:], wkl[:, ec, :], xhs[:],
                                         start=False, stop=(ec == EC - 1))
                    else:
                        xfs = xp.tile([128, 512], F32, tag="xfs")
                        xhs = xp.tile([128, 512], BF, tag="xhs")
                        nc.gpsimd.dma_start(xfs[:], xt_d[es, ts])
                        nc.gpsimd.dma_start(xhs[:], xh_d[es, ts])
                        nc.tensor.matmul(qt_ps[:], wq[:, ec, :], xfs[:],
                                         start=(ec == 0), stop=(ec == EC - 1))
                        nc.tensor.matmul(kt_ps[:], wk[:, ec, :], xfs[:],
                                         start=(ec == 0), stop=(ec == EC - 1))
                    nc.tensor.matmul(vt_ps[:], wv[:, ec, :], xhs[:],
                                     start=(ec == 0), stop=(ec == EC - 1))

                # evacuate; Q/K via hi/lo split (exact residual), V^T -> V
                if use_hilo:
                    nc.scalar.copy(Qh[:, ts], qt_ps[:])
                    nc.vector.tensor_tensor(Ql[:, ts], qt_ps[:], Qh[:, ts],
                                            mybir.AluOpType.subtract)
                    nc.scalar.copy(Kh[:, ts], kt_ps[:])
                    nc.vector.tensor_tensor(Kl[:, ts], kt_ps[:], Kh[:, ts],
                                            mybir.AluOpType.subtract)
                else:
                    nc.scalar.copy(QT[:, ts], qt_ps[:])
                    nc.scalar.copy(KT[:, ts], kt_ps[:])
                vt_sb = evp.tile([128, 512], FP16, tag="vt")
                nc.vector.tensor_copy(vt_sb[:], vt_ps[:])
                # V^T slice [i=128, t=512] -> V[t-inner=128, 4 chunks, i=128]
                nc.sync.dma_start_transpose(V[:, tb8 * 4:(tb8 + 1) * 4, :],
                                            vt_sb[:])

            # ================= Phase 2: attention per (b, head) =================
            for b in range(B):
                for hh in range(HPC):
                    hr = slice(hh * 64, (hh + 1) * 64)
                    PT = bigp.tile([128, T // 128, T], FP16, tag="PT")
                    for tb in range(T // 128):
                        tcols = slice(b * T + tb * 128, b * T + (tb + 1) * 128)
                        s_ps = [psp.tile([128, 512], F32, tag="ps",
                                         name=f"s_ps_{j}")
                                for j in range(4)]
                        if use_hilo:
                            for pi, (lh, rh) in enumerate(
                                    ((Qh, Kh), (Ql, Kh), (Qh, Kl))):
                                for j in range(4):
                                    scols = slice(b * T + j * 512,
                                                  b * T + (j + 1) * 512)
                                    nc.tensor.matmul(
                                        s_ps[j][:], lh[hr, tcols],
                                        rh[hr, scols],
                                        start=(pi == 0), stop=(pi == 2))
                        else:
                            for j in range(4):
                                scols = slice(b * T + j * 512,
                                              b * T + (j + 1) * 512)
                                nc.tensor.matmul(
                                    s_ps[j][:], QT[hr, tcols], KT[hr, scols],
                                    start=True, stop=True)
                        # softmax over the free (s) axis
                        m4 = stp.tile([128, 4], F32, tag="m4")
                        for j in range(4):
                            nc.vector.reduce_max(m4[:, j:j + 1], s_ps[j][:],
                                                 axis=mybir.AxisListType.X)
                        negb = stp.tile([128, 1], F32, tag="negb")
                        nc.vector.reduce_max(negb[:], m4[:],
                                             axis=mybir.AxisListType.X,
                                             negate=True)
                        nc.vector.tensor_scalar_mul(negb[:], negb[:], SCALE)
                        Pt = ptp.tile([128, T], FP16, tag="Pt")
                        d4 = stp.tile([128, 4], F32, tag="d4")
                        for j in range(4):
                            nc.scalar.activation(
                                Pt[:, j * 512:(j + 1) * 512], s_ps[j][:],
                                mybir.ActivationFunctionType.Exp,
                                bias=negb[:], scale=SCALE,
                                accum_out=d4[:, j:j + 1])
                        den = stp.tile([128, 1], F32, tag="den")
                        nc.vector.reduce_sum(den[:], d4[:],
                                             axis=mybir.AxisListType.X)
                        rcp = stp.tile([128, 1], F32, tag="rcp")
                        nc.vector.reciprocal(rcp[:], den[:])
                        nc.vector.tensor_scalar_mul(Pt[:], Pt[:], rcp[:])
                        # P block [t=128, s=T] -> P^T[s-inner, s-chunk, t-cols]
                        nc.sync.dma_start_transpose(PT[:, :, tb * 128:(tb + 1) * 128],
                                                    Pt[:])
                    # PV: O^T[i, t-bank] = sum_s V[s, i] * P^T[s, t].
                    # M=64 per head; col-tile two t-banks into col-groups
                    # (0,0)/(0,64) so the pairs run concurrently on the PE.
                    for nbp in range(2):
                        o_ps0 = psp.tile([64, 512], F32, tag="ps",
                                         name=f"o_ps0_{nbp}")
                        o_ps1 = psp.tile([128, 512], F32, tag="ps",
                                         name=f"o_ps1_{nbp}")
                        nb0, nb1 = 2 * nbp, 2 * nbp + 1
                        for sc in range(T // 128):
                            vsl = V[:, b * (T // 128) + sc, hr]
                            st, sp = (sc == 0), (sc == T // 128 - 1)
                            nc.tensor.matmul(
                                o_ps0[:], vsl,
                                PT[:, sc, nb0 * 512:(nb0 + 1) * 512],
                                start=st, stop=sp, tile_position=(0, 0))
                            nc.tensor.matmul(
                                o_ps1[64:128, :], vsl,
                                PT[:, sc, nb1 * 512:(nb1 + 1) * 512],
                                start=st, stop=sp, tile_position=(0, 64))
                        nc.vector.tensor_copy(
                            OT[hr, b * T + nb0 * 512: b * T + (nb0 + 1) * 512],
                            o_ps0[:])
                        nc.vector.tensor_copy(
                            OT[hr, b * T + nb1 * 512: b * T + (nb1 + 1) * 512],
                            o_ps1[64:128, :])

            # ================= Phase 3: output projection =================
            # out[t, e] = sum_i OT[i, t] * wo[i, e]
            for ob in range(BT // 128):
                trows = slice(ob * 128, (ob + 1) * 128)
                for eb in range(E // 512):
                    w_ps = psp.tile([128, 512], F32, tag="ps")
                    nc.tensor.matmul(w_ps[:], OT[:, trows],
                                     wo[:, eb * 512:(eb + 1) * 512],
                                     start=True, stop=True)
                    o_sb = evp.tile([128, 512], F32, tag="osb")
                    nc.scalar.copy(o_sb[:], w_ps[:])
                    nc.gpsimd.dma_start(out_d[trows, eb * 512:(eb + 1) * 512],
                                        o_sb[:])
    nc.compile()
    return nc


def _split_bf16(a32: np.ndarray):
    hi = a32.astype(BF16)
    lo = (a32 - hi.astype(np.float32)).astype(BF16)
    return hi, lo


def _split_fp16(a32: np.ndarray):
    hi = a32.astype(NF16)
    lo = (a32 - hi.astype(np.float32)).astype(NF16)
    return hi, lo


def make_in_maps(x, Wq, Wk, Wv, Wo, use_hilo: bool = USE_HILO):
    """Build the 8 per-core input maps from the full inputs."""
    x = np.asarray(x, np.float32)
    Wq = np.asarray(Wq, np.float32)
    Wk = np.asarray(Wk, np.float32)
    Wv = np.asarray(Wv, np.float32)
    Wo = np.asarray(Wo, np.float32)

    xt = np.ascontiguousarray(x.reshape(BT, E).T)          # [E, BT]
    xth, xtl = _split_bf16(xt)
    xth16, xtl16 = _split_fp16(xt)
    in_maps = []
    for c in range(NCORES):
        hsl = slice(c * HPC, (c + 1) * HPC)
        # [E, HPC*I] -> [EC, 128, HI]
        def _pmaj(w):  # [E, HI] -> [128, EC, HI] (partition-major)
            return np.ascontiguousarray(
                w.reshape(EC, 128, HI).transpose(1, 0, 2))
        wq_c = _pmaj(np.concatenate(list(Wq[hsl]), axis=1))
        wk_c = _pmaj(np.concatenate(list(Wk[hsl]), axis=1))
        wv_c = _pmaj(np.concatenate(list(Wv[hsl]), axis=1))
        wo_c = np.ascontiguousarray(Wo[:, c * HI:(c + 1) * HI].T)  # [HI, E]
        if use_hilo:
            m = {
                "wv": wv_c.astype(NF16),
                "wo_t": wo_c.astype(NF16),
            }
            m["xh"], m["xl"] = xth16, xtl16
            m["wqh"], m["wql"] = _split_fp16(wq_c)
            m["wkh"], m["wkl"] = _split_fp16(wk_c)
        else:
            m = {
                "wv": wv_c.astype(BF16),
                "wo_t": wo_c.astype(BF16),
            }
            m["xt"] = xt
            m["xh"] = xth
            m["wq"] = wq_c
            m["wk"] = wk_c
        in_maps.append(m)
    return in_maps


_CACHED = {}


def _get_program(use_hilo: bool = USE_HILO) -> bass.Bass:
    if use_hilo not in _CACHED:
        _CACHED[use_hilo] = build_program(use_hilo)
    return _CACHED[use_hilo]


def kernel(**inputs) -> np.ndarray:
    from concourse.bass_utils import run_bass_kernel_spmd

    nc = _get_program()
    in_maps = make_in_maps(inputs["x"], inputs["Wq"], inputs["Wk"],
                           inputs["Wv"], inputs["Wo"])
    res = run_bass_kernel_spmd(nc, in_maps, core_ids=list(range(NCORES)))
    out = np.zeros((BT, E), np.float32)
    for c in range(NCORES):
        out += np.asarray(res.results[c]["out"], np.float32)
    return out.reshape(B, T, E)
